# revision 1
# baseline (speedup 1.0000x reference)
"""Trainium2 Bass kernel for nn_Compressor (NSA-style windowed KV compression).

Math (per reference):
  kv   = x @ wkv_w.T                     [B, S, 1024]
  gate = sigmoid(x @ wgate_w.T)
  kv   = kv * gate + tile(ape)           (ape per position-within-window)
  kv   = mean over windows of 4          [B, S/4, 2, 512]
  out  = norm_w * kv * rsqrt(mean(kv^2, -1) + eps)   [B, S/2, 512]

Distribution: x flattened to [B*S, 4096] = [16384, 4096], sharded into 8
contiguous 2048-row blocks (whole windows per shard); weights replicated.
Each core computes its [1024, 512] output shard; host concatenates.

On-chip strategy (per core):
  - operands cast to fp16 on host (10-bit mantissa: ~6x more accurate than
    bf16 at identical PE throughput; values are O(1) so fp16 range is safe),
    pre-transposed so the contraction dim (D) is the SBUF partition dim;
    accumulation is fp32 in PSUM.
  - both weight matrices resident in SBUF (8 MiB fp16); x streamed once.
  - per (s_tile=128 rows, o_chunk=512 cols): 32+32 accumulating matmuls,
    then sigmoid (ACT) + gate-mul + ape-add (DVE), window-pool via a PE
    matmul against a [128, 32] 0.25-indicator matrix, RMSNorm on the free
    dim, DMA out. Epilogue is deferred by one matmul group so the pool
    matmul never stalls the PE.
  - warmup: weights and block-0 x arrive in fine-grained DMA slices and all
    8 block-0 accumulation groups run dc-outer (the 4th pair borrows the
    idle pool-psum banks), so the PE consumes the weight stream at its
    delivery rate instead of stalling on a full K-pass (timeline model:
    524 -> 469 us).
"""

import sys

sys.path.insert(0, "/opt/trn_rl_repo")

import numpy as np

import concourse.tile as tile
from concourse import bacc, mybir
from concourse.bass_utils import run_bass_kernel_spmd

HALF = np.float16

N_CORES = 8
B, S, D = 4, 4096, 4096
R = 4                  # compress ratio (window)
HD = 512               # head dim
OD = 1024              # coff * head_dim
EPS = 1e-6

ROWS = (B * S) // N_CORES      # 2048 sequence rows per core
DC = D // 128                  # 32 contraction chunks
NT = ROWS // 128               # 16 s-tiles per core
SBLK = 256                     # x columns loaded per DMA block (2 s-tiles)
NW_TILE = 128 // R             # 32 windows per s-tile

_CACHED_NC = None


def _build_nc(reps=1):
    nc = bacc.Bacc("TRN2", target_bir_lowering=False, debug=False,
                   num_devices=N_CORES)
    f32 = mybir.dt.float32
    f16 = mybir.dt.float16

    xt = nc.dram_tensor("xt", [D, ROWS], f16, kind="ExternalInput").ap()
    wkvt = nc.dram_tensor("wkvt", [D, OD], f16, kind="ExternalInput").ap()
    wgt = nc.dram_tensor("wgt", [D, OD], f16, kind="ExternalInput").ap()
    apeb = nc.dram_tensor("apeb", [128, OD], f32, kind="ExternalInput").ap()
    nrmb = nc.dram_tensor("nrmb", [128, HD], f32, kind="ExternalInput").ap()
    poolm = nc.dram_tensor("poolm", [128, NW_TILE], f16, kind="ExternalInput").ap()
    out = nc.dram_tensor("out", [ROWS // R * 2, HD], f32, kind="ExternalOutput").ap()

    # [p, dc, n] views with the contraction dim on partitions
    xt_v = xt.rearrange("(dc p) s -> p dc s", p=128)
    wkvt_v = wkvt.rearrange("(dc p) o -> p dc o", p=128)
    wgt_v = wgt.rearrange("(dc p) o -> p dc o", p=128)
    out_v = out.rearrange("(w two) h -> w two h", two=2)

    with tile.TileContext(nc) as tc:
        with (
            tc.tile_pool(name="const", bufs=1) as const_pool,
            tc.tile_pool(name="wpool", bufs=1) as wpool,
            tc.tile_pool(name="xpool", bufs=2) as xpool,
            tc.tile_pool(name="acts", bufs=2) as acts,
            tc.tile_pool(name="small", bufs=2) as small,
            tc.tile_pool(name="mm", bufs=3, space="PSUM") as psum_pool,
            tc.tile_pool(name="pl", bufs=2, space="PSUM") as pool_psum,
        ):
            WSL = 2   # dc chunks per weight DMA slice
            XSL0 = 8  # dc chunks per block-0 x DMA slice

            # Block 0's x arrives as 4 dc-sliced tiles interleaved with the
            # first weight slices, so the very first matmul only waits for
            # ~1.5 MiB of DMA instead of the whole 4 MiB working set. The
            # remaining weight slices stream behind it.
            xtb0_sl = []
            for s0 in range(DC // XSL0):
                xtb0_part = const_pool.tile([128, XSL0, SBLK], f16,
                                            tag=f"xtb0_{s0}",
                                            name=f"xtb0_{s0}")
                xtb0_sl.append(xtb0_part)
            wkv_sl, wg_sl = [], []

            def load_wslice(s0):
                t = wpool.tile([128, WSL, OD], f16, tag=f"wkv{s0}")
                nc.sync.dma_start(t[:], wkvt_v[:, s0 * WSL:(s0 + 1) * WSL, :])
                wkv_sl.append(t)
                t = wpool.tile([128, WSL, OD], f16, tag=f"wg{s0}")
                nc.sync.dma_start(t[:], wgt_v[:, s0 * WSL:(s0 + 1) * WSL, :])
                wg_sl.append(t)

            for s0 in range(DC // WSL):
                if s0 % ((DC // WSL) // len(xtb0_sl)) == 0:
                    xi = s0 // ((DC // WSL) // len(xtb0_sl))
                    nc.sync.dma_start(
                        xtb0_sl[xi][:],
                        xt_v[:, xi * XSL0:(xi + 1) * XSL0, 0:SBLK])
                load_wslice(s0)

            apeb_sb = const_pool.tile([128, OD], f32)
            nc.sync.dma_start(apeb_sb[:], apeb)
            nrmb_sb = const_pool.tile([128, HD], f32)
            nc.sync.dma_start(nrmb_sb[:], nrmb)
            poolm_sb = const_pool.tile([128, NW_TILE], f16)
            nc.sync.dma_start(poolm_sb[:], poolm)
            eps_sb = const_pool.tile([128, 1], f32)
            nc.gpsimd.memset(eps_sb[:], EPS)

            def load_xblk(blk):
                t = xpool.tile([128, DC, SBLK], f16, tag="xtb")
                nc.sync.dma_start(
                    t[:], xt_v[:, :, blk * SBLK:(blk + 1) * SBLK])
                return t

            def epilogue(ps_kv, ps_g, i, c):
                gate_sb = acts.tile([128, HD], f32, tag="gate")
                nc.scalar.activation(gate_sb[:], ps_g[:],
                                     mybir.ActivationFunctionType.Sigmoid)
                kvg_sb = acts.tile([128, HD], f32, tag="kvg")
                nc.vector.tensor_mul(kvg_sb[:], ps_kv[:], gate_sb[:])
                nc.vector.tensor_add(kvg_sb[:], kvg_sb[:],
                                     apeb_sb[:, c * HD:(c + 1) * HD])
                kvg16 = acts.tile([128, HD], f16, tag="kvg16")
                nc.vector.tensor_copy(kvg16[:], kvg_sb[:])
                pooled_ps = pool_psum.tile([NW_TILE, HD], f32, tag="pooled")
                nc.tensor.matmul(pooled_ps[:], poolm_sb[:], kvg16[:],
                                 start=True, stop=True)
                # RMSNorm over the free (head) dim
                pooled_sb = small.tile([NW_TILE, HD], f32, tag="pooled_sb")
                nc.vector.tensor_copy(pooled_sb[:], pooled_ps[:])
                sqj = small.tile([NW_TILE, HD], f32, tag="sqj")
                ssq = small.tile([NW_TILE, 1], f32, tag="ssq")
                nc.vector.tensor_mul(sqj[:], pooled_sb[:], pooled_sb[:])
                nc.vector.reduce_sum(ssq[:], sqj[:], axis=mybir.AxisListType.X)
                std = small.tile([NW_TILE, 1], f32, tag="std")
                nc.scalar.activation(std[:], ssq[:],
                                     mybir.ActivationFunctionType.Sqrt,
                                     bias=eps_sb[:NW_TILE, :], scale=1.0 / HD)
                rinv = small.tile([NW_TILE, 1], f32, tag="rinv")
                nc.vector.reciprocal(rinv[:], std[:])
                onorm = small.tile([NW_TILE, HD], f32, tag="onorm")
                nc.scalar.mul(onorm[:], pooled_sb[:], rinv[:])
                nc.vector.tensor_mul(onorm[:], onorm[:], nrmb_sb[:NW_TILE, :])
                nc.sync.dma_start(
                    out_v[i * NW_TILE:(i + 1) * NW_TILE, c, :], onorm[:])

            def mm_pair(ps_kv, ps_g, xtb, j, c, dc):
                if isinstance(xtb, list):
                    lhsT = xtb[dc // XSL0][:, dc % XSL0,
                                           j * 128:(j + 1) * 128]
                else:
                    lhsT = xtb[:, dc, j * 128:(j + 1) * 128]
                nc.tensor.matmul(
                    ps_kv[:], lhsT,
                    wkv_sl[dc // WSL][:, dc % WSL, c * HD:(c + 1) * HD],
                    start=(dc == 0), stop=(dc == DC - 1))
                nc.tensor.matmul(
                    ps_g[:], lhsT,
                    wg_sl[dc // WSL][:, dc % WSL, c * HD:(c + 1) * HD],
                    start=(dc == 0), stop=(dc == DC - 1))

            pending = []

            def flush(keep):
                while len(pending) > keep:
                    epilogue(*pending.pop(0))

            for _rep in range(reps):
              for blk in range(NT * 128 // SBLK):
                  if _rep == 0 and blk == 0:
                      xtb = xtb0_sl
                      # Warmup block: run 6 accumulation groups dc-outer so
                      # each arriving weight slice feeds 6 matmuls — keeps the
                      # PE at the weight-DMA delivery rate instead of
                      # stalling on the first full K-pass.
                      groups = []
                      for (jj, cc) in [(0, 0), (0, 1), (1, 0)]:
                          gkv = psum_pool.tile([128, HD], f32, tag="ps_kv")
                          gg = psum_pool.tile([128, HD], f32, tag="ps_g")
                          groups.append((gkv, gg, jj, cc))
                      # 4th pair borrows the (idle during warmup) pool-psum
                      # slots so every weight chunk feeds 8 matmuls
                      gkv = pool_psum.tile([128, HD], f32, tag="pooled")
                      gg = pool_psum.tile([128, HD], f32, tag="pooled")
                      groups.append((gkv, gg, 1, 1))
                      for dc in range(DC):
                          for (gkv, gg, jj, cc) in groups:
                              mm_pair(gkv, gg, xtb, jj, cc, dc)
                      for (gkv, gg, jj, cc) in groups:
                          pending.append((gkv, gg, blk * (SBLK // 128) + jj, cc))
                      rest = []
                  else:
                      xtb = load_xblk(blk)
                      rest = [(jj, cc) for jj in range(SBLK // 128)
                              for cc in range(2)]
                  for (j, c) in rest:
                      i = blk * (SBLK // 128) + j
                      ps_kv = psum_pool.tile([128, HD], f32, tag="ps_kv")
                      ps_g = psum_pool.tile([128, HD], f32, tag="ps_g")
                      for dc in range(DC):
                          mm_pair(ps_kv, ps_g, xtb, j, c, dc)
                      pending.append((ps_kv, ps_g, i, c))
                      flush(1)
            flush(0)

    nc.compile()
    return nc


def _get_nc():
    global _CACHED_NC
    if _CACHED_NC is None:
        _CACHED_NC = _build_nc()
    return _CACHED_NC


def _prep_in_maps(x, wkv_w, wgate_w, ape, norm_w):
    x = np.asarray(x, dtype=np.float32)
    wkv_w = np.asarray(wkv_w, dtype=np.float32)
    wgate_w = np.asarray(wgate_w, dtype=np.float32)
    ape = np.asarray(ape, dtype=np.float32)
    norm_w = np.asarray(norm_w, dtype=np.float32)

    xb = x.reshape(B * S, D).astype(HALF)
    wkvt = np.ascontiguousarray(wkv_w.astype(HALF).T)     # [D, OD]
    wgt = np.ascontiguousarray(wgate_w.astype(HALF).T)    # [D, OD]
    apeb = np.ascontiguousarray(np.tile(ape, (128 // R, 1)))  # [128, OD]
    nrmb = np.ascontiguousarray(np.tile(norm_w[None, :], (128, 1)))  # [128, HD]
    poolm = np.zeros((128, NW_TILE), np.float32)
    poolm[np.arange(128), np.arange(128) // R] = 1.0 / R
    poolm = poolm.astype(HALF)

    in_maps = []
    for k in range(N_CORES):
        xt_k = np.ascontiguousarray(xb[k * ROWS:(k + 1) * ROWS, :].T)  # [D, ROWS]
        in_maps.append({
            "xt": xt_k, "wkvt": wkvt, "wgt": wgt,
            "apeb": apeb, "nrmb": nrmb, "poolm": poolm,
        })
    return in_maps


def kernel(x, wkv_w, wgate_w, ape, norm_w):
    nc = _get_nc()
    in_maps = _prep_in_maps(x, wkv_w, wgate_w, ape, norm_w)
    try:
        res = run_bass_kernel_spmd(nc, in_maps, list(range(N_CORES)))
    except Exception:
        # Transient axon-transport failures are retryable; a wedged device
        # (NRT_EXEC_UNIT_UNRECOVERABLE) recovers with a fresh PJRT session.
        try:
            import jax
            jax.clear_backends()
        except Exception:
            pass
        res = run_bass_kernel_spmd(nc, in_maps, list(range(N_CORES)))
    shards = [res.results[k]["out"] for k in range(N_CORES)]
    return np.concatenate(shards, axis=0).reshape(B, S // R * 2, HD)

